# revision 6
# baseline (speedup 1.0000x reference)
"""Trainium2 Bass kernel for nn_MixtureOfExpertsModel (moe_routing).

Computes, for x [65536, 1024] and 10 experts with 15 outputs each:
    miu = x @ expert_w^T + expert_b      (per expert)
    xi  = x @ gate_w^T + gate_b          (per expert)
    out = sum_e softmax_e(xi) * miu      -> [65536, 15]

Strategy: pure data parallel over 8 NeuronCores (8192 rows each); at fp16
the kernel is PE-bound (2.52 GMAC/core -> 512 matmuls x ~127.7ns = 65.4us),
so the design keeps the PE streaming 300-column fp16 matmuls back-to-back
and pushes everything else off the critical path:

 * x is repacked on the host into per-slab blocks (512 rows; one contiguous
   8KB run per partition) so every HWDGE load is 128 large descriptors.
 * gate bias folded into the expert weights on the host:
   softmax(xi+gb) ~ exp(gb)*exp(xi), so expert_w' = expert_w * exp(gb) and
   the denominator uses a precomputed E=exp(gb) row vector.  The gate half
   of PSUM is then evicted by the Scalar engine as pe1 = Exp(psum) while
   the Vector engine evicts the expert half (psum + bias -> fp16), in
   parallel.
 * head: sync ring carries slab-0's first half then the slab stream;
   scalar ring carries the merged weights+bias+E tensor then slab-0's
   second half.  PE warmup matmuls (memset input, no DMA deps) hold the
   HAM clock gate open until real data lands.
 * processing is per 2-subtile psum unit (2 banks, 3 bufs) with
   num=mx*pe1 / den=E*pe1 muls and a segmented reduce over experts; group
   finals (reciprocal + mul) per 4-slab group, stores are 128 x 960B
   descriptors (rows permuted on host: partition p owns rows p*16+s).
 * tail: the last slab runs as units (0,2),(2,1),(3,1) (1-bank psum pool);
   the last group's rows for subtiles 0..14 are stored right after the
   (2,1) unit's finals (900B/partition), and the final 128-row subtile is
   transposed (DVE 32x32 blocks) and stored as 32 x 512B descriptors to a
   separate out2 tensor that the host stitches back.
"""

import sys

if "/opt/trn_rl_repo" not in sys.path:
    sys.path.insert(0, "/opt/trn_rl_repo")

import numpy as np

import concourse.bass as bass
import concourse.bacc as bacc
import concourse.tile as tile
import concourse.mybir as mybir
from concourse.bass_utils import run_bass_kernel_spmd

F32 = mybir.dt.float32
FP16 = mybir.dt.float16

MDT = FP16
NPDT = np.float16

BS = 65536
K = 1024
E = 10
O = 15
EO = E * O                # 150
NCOL = 2 * EO             # 300: cols 0..149 = expert (n=o*E+e), 150..299 = gate
NCORES = 8
RPC = BS // NCORES        # rows per core: 8192
KC = K // 128             # 8 contraction chunks
SLAB = 512                # rows per slab = 4 matmul subtiles
NSUB = SLAB // 128        # 4 subtiles per slab
NSLAB = RPC // SLAB       # 16 slabs per core
GROUP = 4                 # slabs per output group (2048 rows per out DMA)
NGRP = NSLAB // GROUP
PREFETCH = 5              # x slabs in flight ahead of compute
N_WARMUP = 20             # PE warmup matmuls (HAM clock-gate release)
WEXT = KC * NCOL          # 2400: weight cols per partition
WTOT = WEXT + EO + EO     # + expert bias (fp16) + E=exp(gate_b) (fp16)


def _build():
    nc = bacc.Bacc("TRN2", target_bir_lowering=False, debug=False,
                   num_devices=NCORES)
    # xt row k*128+q holds slab k's contiguous (j, c, m) block: j = subtile,
    # c = k-chunk, m = moving-row index p; q = k-chunk partition.
    xt = nc.dram_tensor("xt", [NSLAB * 128, NSUB * KC * 128], MDT,
                        kind="ExternalInput").ap()
    # wtb: [wt (2400) | expert-bias' (150) | E=exp(gate_b) (150)] per
    # partition; wt[q, (c, n)] = w'[n, c*128+q]; bias/E replicated.
    wtb = nc.dram_tensor("wtb", [128, WTOT], MDT, kind="ExternalInput").ap()
    out = nc.dram_tensor("out", [RPC, O], F32, kind="ExternalOutput").ap()
    # final subtile, transposed: out2[o, p] = row (last_g0 + p*16 + 15), o<15
    out2 = nc.dram_tensor("out2", [32, 128], F32, kind="ExternalOutput").ap()

    with tile.TileContext(nc) as tc:
        with (
            tc.tile_pool(name="const", bufs=1) as cp,
            tc.tile_pool(name="x0", bufs=1) as x0p,
            tc.tile_pool(name="x", bufs=PREFETCH + 2) as xp,
            tc.tile_pool(name="ps", bufs=3, space="PSUM") as ps_pool,
            tc.tile_pool(name="pst", bufs=2, space="PSUM") as pst_pool,
            tc.tile_pool(name="mx", bufs=4) as mx_pool,
            tc.tile_pool(name="pe", bufs=6) as pe_pool,
            tc.tile_pool(name="pd", bufs=4) as pd_pool,
            tc.tile_pool(name="nd", bufs=2) as nd_pool,
            tc.tile_pool(name="ob", bufs=2) as ob_pool,
        ):
            HX = 2 * KC * 128     # half-slab elements per partition

            # Sync ring: both slab-0 halves, then the slab stream.
            s0a = x0p.tile([128, HX], MDT, name="s0a")
            nc.sync.dma_start(s0a[:], xt[0:128, 0:HX])
            s0b = x0p.tile([128, HX], MDT, name="s0b")
            nc.sync.dma_start(s0b[:], xt[0:128, HX:2 * HX])
            # Scalar ring: merged weights+bias+E (plus output stores later).
            wt_t = cp.tile([128, WTOT], MDT, name="wt_t")
            nc.scalar.dma_start(wt_t[:], wtb[:])

            wt_v = wt_t[:, 0:WEXT].rearrange("p (c n) -> p c n", c=KC)

            def wslice(c):
                return wt_v[:, c, :]

            s0v = [s0a[:].rearrange("p (j c m) -> p j c m", j=2, c=KC),
                   s0b[:].rearrange("p (j c m) -> p j c m", j=2, c=KC)]

            xts = {}
            for k in range(1, min(1 + PREFETCH, NSLAB)):
                xt_t = xp.tile([128, NSUB * KC * 128], MDT, tag="xt",
                               name=f"xt_{k}")
                nc.sync.dma_start(xt_t[:], xt[k * 128:(k + 1) * 128, :])
                xts[k] = xt_t

            # Expert bias (fp32 upcast) and E rows broadcast to both
            # subtile planes of a unit -- on DVE so the Scalar queue has no
            # activation before its first Exp.
            biasb_t = cp.tile([128, 2, EO], F32, name="biasb_t")
            e2_t = cp.tile([128, 2, EO], MDT, name="e2_t")
            for j in range(2):
                nc.vector.tensor_copy(biasb_t[:, j, :],
                                      wt_t[:, WEXT:WEXT + EO])
                nc.vector.tensor_copy(e2_t[:, j, :],
                                      wt_t[:, WEXT + EO:WEXT + 2 * EO])
            # Padded final-unit output tile (cols 15:32 must be defined for
            # the 32x32 transpose).
            obL = cp.tile([128, 32], F32, name="obL")
            nc.vector.memset(obL[:], 0.0)
            trL = cp.tile([32, 128], F32, name="trL")

            # Warm up the PE's HAM clock gate while the weights and slab 0
            # stream in: matmuls on a memset tile, no DMA deps.
            wu_in = cp.tile([128, NCOL], MDT, name="wu_in")
            nc.gpsimd.memset(wu_in[:], 0.125)
            wu_ps = pst_pool.tile([128, 512], F32, tag="pst", name="wu_ps")
            for _ in range(N_WARMUP):
                nc.tensor.matmul(
                    wu_ps[:, 0:NCOL], wu_in[:, 0:128], wu_in[:],
                    start=True, stop=True, skip_group_check=True,
                )

            def stat(k, j, c):
                if k == 0:
                    return s0v[j // 2][:, j % 2, c, :]
                return xts[k][:].rearrange(
                    "p (j c m) -> p j c m", j=NSUB, c=KC)[:, j, c, :]

            def post_unit(k, j0, nj, psj, ndb, kin):
                """Evict psum unit (j0..j0+nj), exp, muls, segmented reduce."""
                mx = mx_pool.tile([128, nj, EO], MDT, tag="mx",
                                  name=f"mx_{k}_{j0}", padded_shape=[128, 2, EO])
                pe1 = pe_pool.tile([128, nj, EO], MDT, tag="pe",
                                   name=f"pe_{k}_{j0}", padded_shape=[128, 2, EO])
                pd = pd_pool.tile([128, 2, nj, EO], MDT, tag="pd",
                                  name=f"pd_{k}_{j0}",
                                  padded_shape=[128, 2, 2, EO])
                pv = psj[:].rearrange("p (s b) -> p s b", s=nj)
                # Vector: expert half + bias -> fp16 (psum reader #1)
                nc.vector.tensor_add(mx[:], pv[:, :, 0:EO],
                                     biasb_t[:, 0:nj, :])
                # Scalar: gate half exp straight from psum (psum reader #2)
                nc.scalar.activation(pe1[:], pv[:, :, EO:NCOL],
                                     mybir.ActivationFunctionType.Exp)
                # num = (miu*E) * exp(xi); den = E * exp(xi).  Near the tail
                # the den mul goes to the otherwise-idle GpSimd so the DVE
                # queue stays short (except the very last unit: a serial DVE
                # chain avoids a cross-engine sync on the critical tail).
                nc.vector.tensor_mul(pd[:, 0, :, :], mx[:], pe1[:])
                den_eng = (nc.gpsimd if (k >= NSLAB - 2 and not (j0 == 3))
                           else nc.vector)
                den_eng.tensor_mul(pd[:, 1, :, :], e2_t[:, 0:nj, :], pe1[:])
                nc.vector.reduce_sum(
                    ndb[:, :, kin * NSUB + j0:kin * NSUB + j0 + nj, :],
                    pd[:].rearrange("p h s (o e) -> p (h s) o e", o=O),
                    axis=mybir.AxisListType.X,
                )

            ob = None
            ndb = None
            for k in range(NSLAB):
                kin = k % GROUP
                g = k // GROUP
                last_slab = (k == NSLAB - 1)
                if kin == 0:
                    ob = ob_pool.tile([128, GROUP * NSUB * O], F32, tag="ob",
                                      name=f"ob_{g}")
                    # h-major: ndb[:, 0, :] = num plane, ndb[:, 1, :] = den.
                    ndb = nd_pool.tile([128, 2, GROUP * NSUB, O], F32,
                                       tag="ndb", name=f"ndb_{g}")
                kp = k + 1 + PREFETCH
                if kp < NSLAB:
                    xt_t = xp.tile([128, NSUB * KC * 128], MDT, tag="xt",
                                   name=f"xt_{kp}")
                    nc.sync.dma_start(xt_t[:], xt[kp * 128:(kp + 1) * 128, :])
                    xts[kp] = xt_t

                units = [(0, 2), (2, 1), (3, 1)] if last_slab else \
                        [(0, 2), (2, 2)]
                for (j0, nj) in units:
                    pool = pst_pool if last_slab and nj == 1 else ps_pool
                    psj = pool.tile([128, nj * 512], F32,
                                    tag="pst" if pool is pst_pool else "ps",
                                    name=f"ps_{k}_{j0}",
                                    padded_shape=[128, 2 * 512]
                                    if pool is ps_pool else None)
                    for jj in range(nj):
                        for c in range(KC):
                            nc.tensor.matmul(
                                psj[:, jj * 512:jj * 512 + NCOL],
                                stat(k, j0 + jj, c), wslice(c),
                                start=(c == 0), stop=(c == KC - 1),
                            )
                    post_unit(k, j0, nj, psj, ndb, kin)
                    if last_slab and (j0, nj) == (2, 1):
                        # Finals + store for group subtiles 0..14 while the
                        # final subtile's matmuls stream.
                        rden = nd_pool.tile([128, GROUP * NSUB * O], F32,
                                            tag="rden", name=f"rden_{g}")
                        cut = (GROUP * NSUB - 1) * O
                        nc.vector.reciprocal_approx_fast(
                            rden[:, 0:cut],
                            ndb[:, 1, 0:GROUP * NSUB - 1, :]
                            .rearrange("p s o -> p (s o)"))
                        nc.gpsimd.tensor_mul(
                            ob[:, 0:cut],
                            ndb[:, 0, 0:GROUP * NSUB - 1, :]
                            .rearrange("p s o -> p (s o)"),
                            rden[:, 0:cut])
                        g0 = g * GROUP * SLAB
                        nc.scalar.dma_start(
                            out[g0:g0 + GROUP * SLAB, :]
                            .rearrange("(p s) o -> p (s o)", p=128)[:, 0:cut],
                            ob[:, 0:cut],
                        )
                if kin == GROUP - 1:
                    if last_slab:
                        # Final 128-row subtile: finals, 32x32-block
                        # transpose, 32-descriptor store.
                        nc.vector.reciprocal_approx_fast(
                            obL[:, 16:16 + O],
                            ndb[:, 1, GROUP * NSUB - 1, :])
                        nc.vector.tensor_mul(
                            obL[:, 0:O],
                            ndb[:, 0, GROUP * NSUB - 1, :],
                            obL[:, 16:16 + O])
                        for b in range(4):
                            nc.vector.transpose(
                                trL[0:32, b * 32:(b + 1) * 32],
                                obL[b * 32:(b + 1) * 32, 0:32])
                        nc.sync.dma_start(out2[:], trL[:])
                    else:
                        # Per-group finals; num*rden on the idle GpSimd.
                        rden = nd_pool.tile([128, GROUP * NSUB * O], F32,
                                            tag="rden", name=f"rden_{g}")
                        nc.vector.reciprocal_approx_fast(
                            rden[:],
                            ndb[:, 1, :, :].rearrange("p s o -> p (s o)"))
                        nc.gpsimd.tensor_mul(
                            ob[:],
                            ndb[:, 0, :, :].rearrange("p s o -> p (s o)"),
                            rden[:])
                        g0 = g * GROUP * SLAB
                        # rows r = g0 + p*16 + s (host permutes x to match)
                        nc.scalar.dma_start(
                            out[g0:g0 + GROUP * SLAB, :]
                            .rearrange("(p s) o -> p (s o)", p=128),
                            ob[:],
                        )
    nc.compile()
    return nc


_NC = None


def _get_nc():
    global _NC
    if _NC is None:
        _NC = _build()
    return _NC


def _prep_inputs(x, expert_w, expert_b, gate_w, gate_b):
    # Fold exp(gate_b) into the expert path: softmax(xi+gb) ~ E*exp(xi)
    # with E = exp(gb); num = sum_e (miu*E)*exp(xi), den = sum_e E*exp(xi).
    ew = np.asarray(expert_w, np.float64).reshape(E, O, K)
    eb = np.asarray(expert_b, np.float64).reshape(E, O)
    gw = np.asarray(gate_w, np.float64).reshape(E, O, K)
    gb = np.asarray(gate_b, np.float64).reshape(E, O)
    # Stabilize: subtract per-output max of gb (exactly like softmax shift;
    # cancels in num/den).  Keeps E in [exp(-range), 1].
    gshift = gb.max(axis=0, keepdims=True)
    Ee = np.exp(gb - gshift)                     # (E, O)
    ewp = ew * Ee[:, :, None]
    ebp = eb * Ee
    # o-major columns (n = o*E + e) so the segmented reduce over experts
    # reads contiguous runs.
    w = np.concatenate([
        ewp.transpose(1, 0, 2).reshape(EO, K),
        gw.transpose(1, 0, 2).reshape(EO, K),
    ], axis=0)                                   # [300, K], col n = o*E + e
    be = ebp.T.reshape(EO)                       # expert bias', o-major
    ee = Ee.T.reshape(EO)                        # E, o-major
    # wt[q, (c, n)] = w[n, c*128+q]
    wt = w.reshape(NCOL, KC, 128).transpose(2, 1, 0).reshape(128, KC * NCOL)
    wtb = np.concatenate([
        wt,
        np.broadcast_to(be, (128, EO)),
        np.broadcast_to(ee, (128, EO)),
    ], axis=1).astype(NPDT)
    wtb = np.ascontiguousarray(wtb)
    # Row permutation: within each 2048-row group g of a core, partition p
    # owns rows g*2048 + p*16 + kin*4 + j (slab k = g*4+kin, subtile j).
    # Moving-row index m = p; block layout per slab-row q is (j, c, m).
    x16 = np.asarray(x).astype(NPDT)
    arr = x16.reshape(NCORES, NGRP, 128, GROUP, NSUB, KC, 128)
    #                 core    g     p    kin    j    c   q
    xt = np.ascontiguousarray(arr.transpose(0, 1, 3, 6, 4, 5, 2)) \
        .reshape(NCORES, NSLAB * 128, NSUB * KC * 128)
    in_maps = [{"xt": xt[i], "wtb": wtb} for i in range(NCORES)]
    return in_maps


def _run(in_maps, **kw):
    res = run_bass_kernel_spmd(
        _get_nc(), in_maps, core_ids=list(range(NCORES)), **kw)
    outs = []
    for r in res.results:
        o = np.array(r["out"])          # [RPC, 15]; s=15 rows of the last
        o2 = np.array(r["out2"])        # group come from out2 instead
        g0 = (NGRP - 1) * GROUP * SLAB
        o[g0 + 15::16, :] = o2[0:O, :].T
        outs.append(o)
    out = np.concatenate(outs, axis=0)
    return out, res


def kernel(x, expert_w, expert_b, gate_w, gate_b):
    in_maps = _prep_inputs(x, expert_w, expert_b, gate_w, gate_b)
    out, _ = _run(in_maps)
    return out


def kernel_traced(x, expert_w, expert_b, gate_w, gate_b, **kw):
    """Like kernel() but returns (out, BassKernelResults) with an NTFF trace."""
    in_maps = _prep_inputs(x, expert_w, expert_b, gate_w, gate_b)
    return _run(in_maps, trace=True, **kw)
